# revision 39
# baseline (speedup 1.0000x reference)
"""Trainium2 Bass kernel for nn_LowpassDetector.

Computes: power = re^2 + im^2, 5-tap FIR (b), order-4 IIR recurrence (a)
along time, for signal [2, T=16384, B=2048] -> y [T, B].

Strategy: the FIR+IIR cascade is LTI with all poles at radius <= 0.758,
so the combined impulse response decays below 1e-15 within 128 taps.
The whole filter is therefore exactly (to fp32) a block-Toeplitz matmul:
  y_blk[b] = T0 @ x_blk[b] + T1 @ x_blk[b-1]     (b >= 1)
  y_blk[0] = L0 @ x_blk[0]
where L0 is the exact 128x128 operator of the reference recurrence
(including its nonstandard "first 5 samples pass through" initial
condition), built on the host in float64 by running the reference on
basis vectors. Channels (2048) are sharded 256 per core across 8 cores;
time blocks of 128 map to the TensorEngine contraction dim.

The kernel is HBM-bound: 48 MB/core, ~134 us at 358 GB/s.

v8:
- The host pre-transposes each core's input into the kernel's natural
  layout so every chunk load is ONE fully-contiguous DMA (one DRAM
  region per chunk, partition lines back-to-back; strided layouts with
  128-256 KB row strides regressed DMA ~15 us).
- Input DMAs run on the GpSimd SWDGE queue and CAST f32 -> f16 in the
  DMA datapath: HBM bytes unchanged, but all elementwise work drops to
  16-bit. Squares+adds run on the Vector engine at its 2x 16-bit rate
  (~0.6 us/op vs ~1.05 on Scalar), removing the Scalar-throughput
  backbone that paced every drain tail. fp16 input rounding adds
  ~1e-3 relative error, well under the 2e-2 tolerance.
- Output stores run on the Sync HWDGE queue; each queue's sem waits
  then only gate its own stream (no load-behind-store head-of-line).
- Time blocks are processed in variable-width chunks
  [2,4, 8*14, 4,4,2]: small first chunk = early first bytes; small
  final chunks = short compute tail after the input stream ends.
- PSUM drains all run on the (otherwise idle) Scalar engine and are
  software-pipelined one half-chunk behind their matmuls, so no
  in-order stream ever stalls waiting on the TensorEngine.
- Matmuls run in fp16 (weights rounded to fp16): 1 col/cycle.
"""

import sys
from contextlib import ExitStack

import numpy as np

for _p in ("/opt/trn_rl_repo",):
    if _p not in sys.path:
        sys.path.insert(0, _p)

import concourse.bass as bass  # noqa: E402
import concourse.tile as tile  # noqa: E402
from concourse import bacc, mybir  # noqa: E402
from concourse.bass_utils import run_bass_kernel_spmd  # noqa: E402

T, B, NCORES = 16384, 2048, 8
BL = 128                # time-block size (= PE contraction dim)
NB = T // BL            # 128 time blocks
C = B // NCORES         # 256 channels per core
F32 = mybir.dt.float32
F16 = mybir.dt.float16
COPY = mybir.ActivationFunctionType.Copy

# chunk widths (in 128-step time blocks)
WLIST = [2, 4] + [8] * 14 + [4, 4, 2]
assert sum(WLIST) == NB

TRACE = False           # set by test harness for NTFF profiling
LAST_RESULTS = None     # BassKernelResults of the last run (for profiling)

_program_cache = {}


def _reference_operator(bb, aa, n):
    """Exact linear operator of the reference filter on n samples (float64).

    Columns are responses to basis vectors; replicates the reference
    semantics: xf = zero-padded cross-correlation with b, first 5 outputs
    pass through, recurrence y[t] = xf[t] - sum_j a_j y[t-j] from t=5.
    """
    x = np.eye(n)
    xp = np.concatenate([np.zeros((4, n)), x], 0)
    xf = sum(bb[k] * xp[k:k + n] for k in range(5))
    y = xf.copy()
    at = aa[:4]
    for t in range(5, n):
        y[t] = xf[t] - (at[0] * y[t - 4] + at[1] * y[t - 3]
                        + at[2] * y[t - 2] + at[3] * y[t - 1])
    return y


def _build_wpack(b32, a32):
    """[128, 3*128] fp16: [L0.T | T0.T | T1.T] stationary matmul operands."""
    bb = np.asarray(b32, np.float64)
    aa = np.asarray(a32, np.float64)
    M = _reference_operator(bb, aa, 3 * BL)
    L0 = M[0:BL, 0:BL]
    T0 = M[2 * BL:3 * BL, 2 * BL:3 * BL]
    T1 = M[2 * BL:3 * BL, BL:2 * BL]
    # truncation + init-transient leakage must be below fp32 noise
    leak = np.abs(M[2 * BL:3 * BL, 0:BL]).max()
    dev = max(np.abs(M[BL:2 * BL, BL:2 * BL] - T0).max(),
              np.abs(M[BL:2 * BL, 0:BL] - T1).max())
    assert leak < 1e-9 and dev < 1e-9, (leak, dev)
    return np.concatenate(
        [np.ascontiguousarray(W.T).astype(np.float16) for W in (L0, T0, T1)],
        axis=1)


def _build_program():
    nc = bacc.Bacc("TRN2", target_bir_lowering=False, debug=False)
    x_ds = [nc.dram_tensor(f"x{j}", [BL, 2 * W * C], F32,
                           kind="ExternalInput").ap()
            for j, W in enumerate(WLIST)]
    w_d = nc.dram_tensor("w", [BL, 3 * BL], F16, kind="ExternalInput").ap()
    y_ds = [nc.dram_tensor(f"y{j}", [BL, W * C], F32,
                           kind="ExternalOutput").ap()
            for j, W in enumerate(WLIST)]

    with tile.TileContext(nc) as tc, ExitStack() as ctx:
        wpool = ctx.enter_context(tc.tile_pool(name="w", bufs=1))
        wt = wpool.tile([BL, 3 * BL], F16, tag="w", name="wt")
        nc.gpsimd.dma_start(wt[:], w_d)
        WL0 = wt[:, 0:BL]
        WT0 = wt[:, BL:2 * BL]
        WT1 = wt[:, 2 * BL:3 * BL]

        iopool = ctx.enter_context(tc.tile_pool(name="io", bufs=6))
        hpool = ctx.enter_context(tc.tile_pool(name="h", bufs=4))
        ypool = ctx.enter_context(tc.tile_pool(name="y", bufs=6))
        pspool = ctx.enter_context(tc.tile_pool(name="ps", bufs=4,
                                                space="PSUM"))

        prev_xh = None
        prev_w = 0
        # software-pipelined drains (one half-chunk behind the matmuls)
        # with one coalesced full-chunk store (8 KB partition lines
        # instead of 4 KB: halves store descriptor/packet overhead)
        drainq = []      # (ps_ap, ysb_dst, chunk_rec) awaiting drain

        def pump_drain():
            ps_ap, dst, rec = drainq.pop(0)
            nc.scalar.activation(dst, ps_ap, COPY)
            rec["done"] += 1
            if rec["done"] == rec["need"]:
                cols = rec["cols"]
                nc.sync.dma_start(y_ds[rec["ci"]][:, :cols],
                                  rec["ysb"][:, :cols])

        for ci, W in enumerate(WLIST):
            if ci == 0:
                # first chunk: plain f32 load on the fast-starting Sync
                # HWDGE queue; overlaps the SWDGE (Q7) descriptor-engine
                # cold start that otherwise delays first bytes ~3 us
                xin = iopool.tile([BL, 2 * WLIST[0] * C], F32, tag="xin32")
                nc.sync.dma_start(xin[:], x_ds[0])
            else:
                # SWDGE cast-DMA: f32 in DRAM -> f16 in SBUF
                xin = iopool.tile([BL, 2 * 8 * C], F16, tag="xin")
                nc.gpsimd.dma_start(xin[:, :2 * W * C], x_ds[ci])
            xh = hpool.tile([BL, 8 * C], F16, tag="xh")
            ysb = ypool.tile([BL, 8 * C], F32, tag="ysb")
            rec = {"ysb": ysb, "ci": ci, "cols": W * C,
                   "done": 0, "need": (W + 3) // 4}

            for h in range((W + 3) // 4):
                wh = min(4, W - 4 * h)
                c0 = h * 4 * C
                re_h = xin[:, c0:c0 + wh * C]
                im_h = xin[:, W * C + c0:W * C + c0 + wh * C]
                # elementwise on DVE (fp16 chunks run at 2x throughput)
                nc.vector.tensor_mul(re_h, re_h, re_h)
                nc.vector.tensor_mul(im_h, im_h, im_h)
                nc.vector.tensor_add(xh[:, c0:c0 + wh * C], re_h, im_h)

                ps = pspool.tile([BL, 4 * C], F32, tag="ps")
                for i in range((wh + 1) // 2):
                    pb = 4 * h + 2 * i          # first block of this pair
                    pw = min(2, wh - 2 * i)     # pair width (1 or 2)
                    pp = ps[:, 2 * i * C:(2 * i + pw) * C]
                    cur = xh[:, pb * C:(pb + pw) * C]
                    if pb == 0:
                        if ci == 0:
                            # block 0 of the sequence: exact-init L0
                            nc.tensor.matmul(pp[:, 0:C], WL0, xh[:, 0:C],
                                             start=True, stop=True)
                            if pw == 2:
                                nc.tensor.matmul(pp[:, C:2 * C], WT0,
                                                 xh[:, C:2 * C],
                                                 start=True, stop=False)
                                nc.tensor.matmul(pp[:, C:2 * C], WT1,
                                                 xh[:, 0:C],
                                                 start=False, stop=True)
                        else:
                            # T1 term of block 0 reads the previous
                            # chunk's last block directly
                            nc.tensor.matmul(pp, WT0, cur,
                                             start=True, stop=False)
                            nc.tensor.matmul(
                                pp[:, 0:C], WT1,
                                prev_xh[:, (prev_w - 1) * C:prev_w * C],
                                start=False, stop=True)
                            if pw == 2:
                                nc.tensor.matmul(pp[:, C:2 * C], WT1,
                                                 xh[:, 0:C],
                                                 start=False, stop=True)
                    else:
                        sh = xh[:, (pb - 1) * C:(pb - 1 + pw) * C]
                        nc.tensor.matmul(pp, WT0, cur,
                                         start=True, stop=False)
                        nc.tensor.matmul(pp, WT1, sh,
                                         start=False, stop=True)

                # software-pipelined drain: process the PREVIOUS half now
                # (its matmuls finished ~a half-period ago, so the
                # in-order scalar stream never stalls on PSUM)
                drainq.append((ps[:, :wh * C], ysb[:, c0:c0 + wh * C], rec))
                if len(drainq) > 1:
                    pump_drain()

            prev_xh = xh
            prev_w = W
        while drainq:
            pump_drain()

    nc.compile()
    return nc


def kernel(signal, b, a):
    global LAST_RESULTS
    signal = np.asarray(signal)
    assert signal.shape == (2, T, B), signal.shape

    wpack = _build_wpack(np.asarray(b), np.asarray(a))

    if "prog" not in _program_cache:
        _program_cache["prog"] = _build_program()
    nc = _program_cache["prog"]

    # per-core kernel-natural input layout: chunk j's region holds
    # [p, (i, b, c)] contiguously
    u = np.asarray(signal, np.float32).reshape(2, NB, BL, NCORES, C)
    in_maps = [{"w": wpack} for _ in range(NCORES)]
    b0 = 0
    for j, W in enumerate(WLIST):
        Xc = (u[:, b0:b0 + W].transpose(3, 2, 0, 1, 4)
              .reshape(NCORES, BL, 2 * W * C))
        for k in range(NCORES):
            in_maps[k][f"x{j}"] = Xc[k]
        b0 += W

    res = run_bass_kernel_spmd(nc, in_maps, core_ids=list(range(NCORES)),
                               trace=TRACE)
    LAST_RESULTS = res

    out = np.empty((T, B), np.float32)
    ob = out.reshape(NB, BL, NCORES, C)
    b0 = 0
    for j, W in enumerate(WLIST):
        for k in range(NCORES):
            ob[b0:b0 + W, :, k, :] = (res.results[k][f"y{j}"]
                                      .reshape(BL, W, C).transpose(1, 0, 2))
        b0 += W
    return out


# revision 41
# speedup vs baseline: 1.1391x; 1.1391x over previous
"""Trainium2 Bass kernel for nn_LowpassDetector.

Computes: power = re^2 + im^2, 5-tap FIR (b), order-4 IIR recurrence (a)
along time, for signal [2, T=16384, B=2048] -> y [T, B].

Strategy: the FIR+IIR cascade is LTI with all poles at radius <= 0.758,
so the combined impulse response decays below 1e-15 within 128 taps.
The whole filter is therefore exactly (to fp32) a block-Toeplitz matmul:
  y_blk[b] = T0 @ x_blk[b] + T1 @ x_blk[b-1]     (b >= 1)
  y_blk[0] = L0 @ x_blk[0]
where L0 is the exact 128x128 operator of the reference recurrence
(including its nonstandard "first 5 samples pass through" initial
condition), built on the host in float64 by running the reference on
basis vectors. Channels (2048) are sharded 256 per core across 8 cores;
time blocks of 128 map to the TensorEngine contraction dim.

The kernel is HBM-bound: 48 MB/core, ~134 us at 358 GB/s.

v8:
- The host pre-transposes each core's input into the kernel's natural
  layout so every chunk load is ONE fully-contiguous DMA (one DRAM
  region per chunk, partition lines back-to-back; strided layouts with
  128-256 KB row strides regressed DMA ~15 us).
- Input DMAs run on the GpSimd SWDGE queue and CAST f32 -> f16 in the
  DMA datapath: HBM bytes unchanged, but all elementwise work drops to
  16-bit. Squares+adds run on the Vector engine at its 2x 16-bit rate
  (~0.6 us/op vs ~1.05 on Scalar), removing the Scalar-throughput
  backbone that paced every drain tail. fp16 input rounding adds
  ~1e-3 relative error, well under the 2e-2 tolerance.
- Output stores run on the Sync HWDGE queue; each queue's sem waits
  then only gate its own stream (no load-behind-store head-of-line).
- The output is stored as fp16 (rounded at the PSUM drain) and upcast
  to f32 on the host: halves store-stream HBM bytes for ~5e-4 extra
  relative error (total ~1.1e-3 vs the 2e-2 tolerance).
- Time blocks are processed in variable-width chunks
  [2,4, 8*14, 4,4,2]: small first chunk = early first bytes; small
  final chunks = short compute tail after the input stream ends.
- PSUM drains all run on the (otherwise idle) Scalar engine and are
  software-pipelined one half-chunk behind their matmuls, so no
  in-order stream ever stalls waiting on the TensorEngine.
- Matmuls run in fp16 (weights rounded to fp16): 1 col/cycle.
"""

import sys
from contextlib import ExitStack

import numpy as np

for _p in ("/opt/trn_rl_repo",):
    if _p not in sys.path:
        sys.path.insert(0, _p)

import concourse.bass as bass  # noqa: E402
import concourse.tile as tile  # noqa: E402
from concourse import bacc, mybir  # noqa: E402
from concourse.bass_utils import run_bass_kernel_spmd  # noqa: E402

T, B, NCORES = 16384, 2048, 8
BL = 128                # time-block size (= PE contraction dim)
NB = T // BL            # 128 time blocks
C = B // NCORES         # 256 channels per core
F32 = mybir.dt.float32
F16 = mybir.dt.float16
COPY = mybir.ActivationFunctionType.Copy

# chunk widths (in 128-step time blocks)
WLIST = [2, 4] + [8] * 14 + [4, 4, 2]
assert sum(WLIST) == NB

TRACE = False           # set by test harness for NTFF profiling
LAST_RESULTS = None     # BassKernelResults of the last run (for profiling)

_program_cache = {}


def _reference_operator(bb, aa, n):
    """Exact linear operator of the reference filter on n samples (float64).

    Columns are responses to basis vectors; replicates the reference
    semantics: xf = zero-padded cross-correlation with b, first 5 outputs
    pass through, recurrence y[t] = xf[t] - sum_j a_j y[t-j] from t=5.
    """
    x = np.eye(n)
    xp = np.concatenate([np.zeros((4, n)), x], 0)
    xf = sum(bb[k] * xp[k:k + n] for k in range(5))
    y = xf.copy()
    at = aa[:4]
    for t in range(5, n):
        y[t] = xf[t] - (at[0] * y[t - 4] + at[1] * y[t - 3]
                        + at[2] * y[t - 2] + at[3] * y[t - 1])
    return y


def _build_wpack(b32, a32):
    """[128, 3*128] fp16: [L0.T | T0.T | T1.T] stationary matmul operands."""
    bb = np.asarray(b32, np.float64)
    aa = np.asarray(a32, np.float64)
    M = _reference_operator(bb, aa, 3 * BL)
    L0 = M[0:BL, 0:BL]
    T0 = M[2 * BL:3 * BL, 2 * BL:3 * BL]
    T1 = M[2 * BL:3 * BL, BL:2 * BL]
    # truncation + init-transient leakage must be below fp32 noise
    leak = np.abs(M[2 * BL:3 * BL, 0:BL]).max()
    dev = max(np.abs(M[BL:2 * BL, BL:2 * BL] - T0).max(),
              np.abs(M[BL:2 * BL, 0:BL] - T1).max())
    assert leak < 1e-9 and dev < 1e-9, (leak, dev)
    return np.concatenate(
        [np.ascontiguousarray(W.T).astype(np.float16) for W in (L0, T0, T1)],
        axis=1)


def _build_program():
    nc = bacc.Bacc("TRN2", target_bir_lowering=False, debug=False)
    x_ds = [nc.dram_tensor(f"x{j}", [BL, 2 * W * C], F32,
                           kind="ExternalInput").ap()
            for j, W in enumerate(WLIST)]
    w_d = nc.dram_tensor("w", [BL, 3 * BL], F16, kind="ExternalInput").ap()
    y_ds = [nc.dram_tensor(f"y{j}", [BL, W * C], F16,
                           kind="ExternalOutput").ap()
            for j, W in enumerate(WLIST)]

    with tile.TileContext(nc) as tc, ExitStack() as ctx:
        wpool = ctx.enter_context(tc.tile_pool(name="w", bufs=1))
        wt = wpool.tile([BL, 3 * BL], F16, tag="w", name="wt")
        nc.gpsimd.dma_start(wt[:], w_d)
        WL0 = wt[:, 0:BL]
        WT0 = wt[:, BL:2 * BL]
        WT1 = wt[:, 2 * BL:3 * BL]

        iopool = ctx.enter_context(tc.tile_pool(name="io", bufs=6))
        hpool = ctx.enter_context(tc.tile_pool(name="h", bufs=4))
        ypool = ctx.enter_context(tc.tile_pool(name="y", bufs=6))
        pspool = ctx.enter_context(tc.tile_pool(name="ps", bufs=4,
                                                space="PSUM"))

        prev_xh = None
        prev_w = 0
        pending = None   # (ps_ap, dram_view) of the half awaiting drain

        def emit_drain(pend):
            # drain rounds PSUM f32 -> f16: halves the store stream's HBM
            # bytes (+~5e-4 rel err, tolerance is 2e-2); host upcasts
            ps_ap, dview = pend
            ysb = ypool.tile([BL, 4 * C], F16, tag="ysb")
            dst = ysb[:, :ps_ap.shape[1]]
            nc.scalar.activation(dst, ps_ap, COPY)
            nc.sync.dma_start(dview, dst)

        for ci, W in enumerate(WLIST):
            if ci == 0:
                # first chunk: plain f32 load on the fast-starting Sync
                # HWDGE queue; overlaps the SWDGE (Q7) descriptor-engine
                # cold start that otherwise delays first bytes ~3 us
                xin = iopool.tile([BL, 2 * WLIST[0] * C], F32, tag="xin32")
                nc.sync.dma_start(xin[:], x_ds[0])
            else:
                # SWDGE cast-DMA: f32 in DRAM -> f16 in SBUF
                xin = iopool.tile([BL, 2 * 8 * C], F16, tag="xin")
                nc.gpsimd.dma_start(xin[:, :2 * W * C], x_ds[ci])
            xh = hpool.tile([BL, 8 * C], F16, tag="xh")

            for h in range((W + 3) // 4):
                wh = min(4, W - 4 * h)
                c0 = h * 4 * C
                re_h = xin[:, c0:c0 + wh * C]
                im_h = xin[:, W * C + c0:W * C + c0 + wh * C]
                # elementwise on DVE (fp16 chunks run at 2x throughput)
                nc.vector.tensor_mul(re_h, re_h, re_h)
                nc.vector.tensor_mul(im_h, im_h, im_h)
                nc.vector.tensor_add(xh[:, c0:c0 + wh * C], re_h, im_h)

                ps = pspool.tile([BL, 4 * C], F32, tag="ps")
                for i in range((wh + 1) // 2):
                    pb = 4 * h + 2 * i          # first block of this pair
                    pw = min(2, wh - 2 * i)     # pair width (1 or 2)
                    pp = ps[:, 2 * i * C:(2 * i + pw) * C]
                    cur = xh[:, pb * C:(pb + pw) * C]
                    if pb == 0:
                        if ci == 0:
                            # block 0 of the sequence: exact-init L0
                            nc.tensor.matmul(pp[:, 0:C], WL0, xh[:, 0:C],
                                             start=True, stop=True)
                            if pw == 2:
                                nc.tensor.matmul(pp[:, C:2 * C], WT0,
                                                 xh[:, C:2 * C],
                                                 start=True, stop=False)
                                nc.tensor.matmul(pp[:, C:2 * C], WT1,
                                                 xh[:, 0:C],
                                                 start=False, stop=True)
                        else:
                            # T1 term of block 0 reads the previous
                            # chunk's last block directly
                            nc.tensor.matmul(pp, WT0, cur,
                                             start=True, stop=False)
                            nc.tensor.matmul(
                                pp[:, 0:C], WT1,
                                prev_xh[:, (prev_w - 1) * C:prev_w * C],
                                start=False, stop=True)
                            if pw == 2:
                                nc.tensor.matmul(pp[:, C:2 * C], WT1,
                                                 xh[:, 0:C],
                                                 start=False, stop=True)
                    else:
                        sh = xh[:, (pb - 1) * C:(pb - 1 + pw) * C]
                        nc.tensor.matmul(pp, WT0, cur,
                                         start=True, stop=False)
                        nc.tensor.matmul(pp, WT1, sh,
                                         start=False, stop=True)

                # software-pipelined drain: store the PREVIOUS half now
                # (its matmuls finished ~a half-period ago, so the
                # in-order scalar stream never stalls on PSUM)
                if pending is not None:
                    emit_drain(pending)
                pending = (ps[:, :wh * C], y_ds[ci][:, c0:c0 + wh * C])

            prev_xh = xh
            prev_w = W
        emit_drain(pending)

    nc.compile()
    return nc


def kernel(signal, b, a):
    global LAST_RESULTS
    signal = np.asarray(signal)
    assert signal.shape == (2, T, B), signal.shape

    wpack = _build_wpack(np.asarray(b), np.asarray(a))

    if "prog" not in _program_cache:
        _program_cache["prog"] = _build_program()
    nc = _program_cache["prog"]

    # per-core kernel-natural input layout: chunk j's region holds
    # [p, (i, b, c)] contiguously
    u = np.asarray(signal, np.float32).reshape(2, NB, BL, NCORES, C)
    in_maps = [{"w": wpack} for _ in range(NCORES)]
    b0 = 0
    for j, W in enumerate(WLIST):
        Xc = (u[:, b0:b0 + W].transpose(3, 2, 0, 1, 4)
              .reshape(NCORES, BL, 2 * W * C))
        for k in range(NCORES):
            in_maps[k][f"x{j}"] = Xc[k]
        b0 += W

    res = run_bass_kernel_spmd(nc, in_maps, core_ids=list(range(NCORES)),
                               trace=TRACE)
    LAST_RESULTS = res

    out = np.empty((T, B), np.float32)
    ob = out.reshape(NB, BL, NCORES, C)
    b0 = 0
    for j, W in enumerate(WLIST):
        for k in range(NCORES):
            ob[b0:b0 + W, :, k, :] = (res.results[k][f"y{j}"]
                                      .reshape(BL, W, C).transpose(1, 0, 2))
        b0 += W
    return out


# revision 42
# speedup vs baseline: 1.1442x; 1.0045x over previous
"""Trainium2 Bass kernel for nn_LowpassDetector.

Computes: power = re^2 + im^2, 5-tap FIR (b), order-4 IIR recurrence (a)
along time, for signal [2, T=16384, B=2048] -> y [T, B].

Strategy: the FIR+IIR cascade is LTI with all poles at radius <= 0.758,
so the combined impulse response decays below 1e-15 within 128 taps.
The whole filter is therefore exactly (to fp32) a block-Toeplitz matmul:
  y_blk[b] = T0 @ x_blk[b] + T1 @ x_blk[b-1]     (b >= 1)
  y_blk[0] = L0 @ x_blk[0]
where L0 is the exact 128x128 operator of the reference recurrence
(including its nonstandard "first 5 samples pass through" initial
condition), built on the host in float64 by running the reference on
basis vectors. Channels (2048) are sharded 256 per core across 8 cores;
time blocks of 128 map to the TensorEngine contraction dim.

The kernel is DMA-bound: 32 MB in + 8 MB out per core. Measured floor
~133.7 us = SDMA engine 15's invariant ~120.3 us busy time (constant
regardless of bytes assigned or DGE path; the other 15 engines idle
~21 us against it) + ~13.4 us of runtime-fixed preamble, scope-init
ramp, and wind-down. Device note: timing has degraded phases (+10-17%
on identical code); NEURON_RT_RESET_CORES=1 usually clears them.

Final design (v10):
- The host pre-transposes each core's input into the kernel's natural
  layout so every chunk load is ONE fully-contiguous DMA (one DRAM
  region per chunk, partition lines back-to-back; strided layouts with
  128-256 KB row strides regressed DMA ~15 us).
- Input DMAs run on the GpSimd SWDGE queue and CAST f32 -> f16 in the
  DMA datapath: HBM bytes unchanged, but all elementwise work drops to
  16-bit. Squares+adds run on the Vector engine at its 2x 16-bit rate
  (~0.6 us/op vs ~1.05 on Scalar), removing the Scalar-throughput
  backbone that paced every drain tail. fp16 input rounding adds
  ~1e-3 relative error, well under the 2e-2 tolerance.
- Output stores run on the Sync HWDGE queue; each queue's sem waits
  then only gate its own stream (no load-behind-store head-of-line).
- The output is stored as fp16 (rounded at the PSUM drain) and upcast
  to f32 on the host: halves store-stream HBM bytes for ~5e-4 extra
  relative error (total ~1.1e-3 vs the 2e-2 tolerance).
- Time blocks are processed in variable-width chunks
  [2,4, 8*14, 4,4,2]: small first chunk = early first bytes; small
  final chunks = short compute tail after the input stream ends.
- PSUM drains all run on the (otherwise idle) Scalar engine and are
  software-pipelined one half-chunk behind their matmuls, so no
  in-order stream ever stalls waiting on the TensorEngine.
- Matmuls run in fp16 (weights rounded to fp16): 1 col/cycle.
"""

import sys
from contextlib import ExitStack

import numpy as np

for _p in ("/opt/trn_rl_repo",):
    if _p not in sys.path:
        sys.path.insert(0, _p)

import concourse.bass as bass  # noqa: E402
import concourse.tile as tile  # noqa: E402
from concourse import bacc, mybir  # noqa: E402
from concourse.bass_utils import run_bass_kernel_spmd  # noqa: E402

T, B, NCORES = 16384, 2048, 8
BL = 128                # time-block size (= PE contraction dim)
NB = T // BL            # 128 time blocks
C = B // NCORES         # 256 channels per core
F32 = mybir.dt.float32
F16 = mybir.dt.float16
COPY = mybir.ActivationFunctionType.Copy

# chunk widths (in 128-step time blocks)
WLIST = [2, 4] + [8] * 14 + [4, 4, 2]
assert sum(WLIST) == NB

TRACE = False           # set by test harness for NTFF profiling
LAST_RESULTS = None     # BassKernelResults of the last run (for profiling)

_program_cache = {}


def _reference_operator(bb, aa, n):
    """Exact linear operator of the reference filter on n samples (float64).

    Columns are responses to basis vectors; replicates the reference
    semantics: xf = zero-padded cross-correlation with b, first 5 outputs
    pass through, recurrence y[t] = xf[t] - sum_j a_j y[t-j] from t=5.
    """
    x = np.eye(n)
    xp = np.concatenate([np.zeros((4, n)), x], 0)
    xf = sum(bb[k] * xp[k:k + n] for k in range(5))
    y = xf.copy()
    at = aa[:4]
    for t in range(5, n):
        y[t] = xf[t] - (at[0] * y[t - 4] + at[1] * y[t - 3]
                        + at[2] * y[t - 2] + at[3] * y[t - 1])
    return y


def _build_wpack(b32, a32):
    """[128, 3*128] fp16: [L0.T | T0.T | T1.T] stationary matmul operands."""
    bb = np.asarray(b32, np.float64)
    aa = np.asarray(a32, np.float64)
    M = _reference_operator(bb, aa, 3 * BL)
    L0 = M[0:BL, 0:BL]
    T0 = M[2 * BL:3 * BL, 2 * BL:3 * BL]
    T1 = M[2 * BL:3 * BL, BL:2 * BL]
    # truncation + init-transient leakage must be below fp32 noise
    leak = np.abs(M[2 * BL:3 * BL, 0:BL]).max()
    dev = max(np.abs(M[BL:2 * BL, BL:2 * BL] - T0).max(),
              np.abs(M[BL:2 * BL, 0:BL] - T1).max())
    assert leak < 1e-9 and dev < 1e-9, (leak, dev)
    return np.concatenate(
        [np.ascontiguousarray(W.T).astype(np.float16) for W in (L0, T0, T1)],
        axis=1)


def _build_program():
    nc = bacc.Bacc("TRN2", target_bir_lowering=False, debug=False)
    x_ds = [nc.dram_tensor(f"x{j}", [BL, 2 * W * C], F32,
                           kind="ExternalInput").ap()
            for j, W in enumerate(WLIST)]
    w_d = nc.dram_tensor("w", [BL, 3 * BL], F16, kind="ExternalInput").ap()
    y_ds = [nc.dram_tensor(f"y{j}", [BL, W * C], F16,
                           kind="ExternalOutput").ap()
            for j, W in enumerate(WLIST)]

    with tile.TileContext(nc) as tc, ExitStack() as ctx:
        wpool = ctx.enter_context(tc.tile_pool(name="w", bufs=1))
        wt = wpool.tile([BL, 3 * BL], F16, tag="w", name="wt")
        nc.gpsimd.dma_start(wt[:], w_d)
        WL0 = wt[:, 0:BL]
        WT0 = wt[:, BL:2 * BL]
        WT1 = wt[:, 2 * BL:3 * BL]

        iopool = ctx.enter_context(tc.tile_pool(name="io", bufs=6))
        hpool = ctx.enter_context(tc.tile_pool(name="h", bufs=4))
        ypool = ctx.enter_context(tc.tile_pool(name="y", bufs=6))
        pspool = ctx.enter_context(tc.tile_pool(name="ps", bufs=4,
                                                space="PSUM"))

        prev_xh = None
        prev_w = 0
        pending = None   # (ps_ap, dram_view) of the half awaiting drain

        def emit_drain(pend):
            # drain rounds PSUM f32 -> f16: halves the store stream's HBM
            # bytes (+~5e-4 rel err, tolerance is 2e-2); host upcasts
            ps_ap, dview = pend
            ysb = ypool.tile([BL, 4 * C], F16, tag="ysb")
            dst = ysb[:, :ps_ap.shape[1]]
            nc.scalar.activation(dst, ps_ap, COPY)
            nc.sync.dma_start(dview, dst)

        for ci, W in enumerate(WLIST):
            if ci == 0:
                # first chunk: plain f32 load on the fast-starting Sync
                # HWDGE queue; overlaps the SWDGE (Q7) descriptor-engine
                # cold start that otherwise delays first bytes ~3 us
                xin = iopool.tile([BL, 2 * WLIST[0] * C], F32, tag="xin32")
                nc.sync.dma_start(xin[:], x_ds[0])
            else:
                # SWDGE cast-DMA: f32 in DRAM -> f16 in SBUF
                xin = iopool.tile([BL, 2 * 8 * C], F16, tag="xin")
                nc.gpsimd.dma_start(xin[:, :2 * W * C], x_ds[ci])
            xh = hpool.tile([BL, 8 * C], F16, tag="xh")

            for h in range((W + 3) // 4):
                wh = min(4, W - 4 * h)
                c0 = h * 4 * C
                re_h = xin[:, c0:c0 + wh * C]
                im_h = xin[:, W * C + c0:W * C + c0 + wh * C]
                # elementwise on DVE (fp16 chunks run at 2x throughput)
                nc.vector.tensor_mul(re_h, re_h, re_h)
                nc.vector.tensor_mul(im_h, im_h, im_h)
                nc.vector.tensor_add(xh[:, c0:c0 + wh * C], re_h, im_h)

                ps = pspool.tile([BL, 4 * C], F32, tag="ps")
                for i in range((wh + 1) // 2):
                    pb = 4 * h + 2 * i          # first block of this pair
                    pw = min(2, wh - 2 * i)     # pair width (1 or 2)
                    pp = ps[:, 2 * i * C:(2 * i + pw) * C]
                    cur = xh[:, pb * C:(pb + pw) * C]
                    if pb == 0:
                        if ci == 0:
                            # block 0 of the sequence: exact-init L0
                            nc.tensor.matmul(pp[:, 0:C], WL0, xh[:, 0:C],
                                             start=True, stop=True)
                            if pw == 2:
                                nc.tensor.matmul(pp[:, C:2 * C], WT0,
                                                 xh[:, C:2 * C],
                                                 start=True, stop=False)
                                nc.tensor.matmul(pp[:, C:2 * C], WT1,
                                                 xh[:, 0:C],
                                                 start=False, stop=True)
                        else:
                            # T1 term of block 0 reads the previous
                            # chunk's last block directly
                            nc.tensor.matmul(pp, WT0, cur,
                                             start=True, stop=False)
                            nc.tensor.matmul(
                                pp[:, 0:C], WT1,
                                prev_xh[:, (prev_w - 1) * C:prev_w * C],
                                start=False, stop=True)
                            if pw == 2:
                                nc.tensor.matmul(pp[:, C:2 * C], WT1,
                                                 xh[:, 0:C],
                                                 start=False, stop=True)
                    else:
                        sh = xh[:, (pb - 1) * C:(pb - 1 + pw) * C]
                        nc.tensor.matmul(pp, WT0, cur,
                                         start=True, stop=False)
                        nc.tensor.matmul(pp, WT1, sh,
                                         start=False, stop=True)

                # software-pipelined drain: store the PREVIOUS half now
                # (its matmuls finished ~a half-period ago, so the
                # in-order scalar stream never stalls on PSUM)
                if pending is not None:
                    emit_drain(pending)
                pending = (ps[:, :wh * C], y_ds[ci][:, c0:c0 + wh * C])

            prev_xh = xh
            prev_w = W
        emit_drain(pending)

    nc.compile()
    return nc


def kernel(signal, b, a):
    global LAST_RESULTS
    signal = np.asarray(signal)
    assert signal.shape == (2, T, B), signal.shape

    wpack = _build_wpack(np.asarray(b), np.asarray(a))

    if "prog" not in _program_cache:
        _program_cache["prog"] = _build_program()
    nc = _program_cache["prog"]

    # per-core kernel-natural input layout: chunk j's region holds
    # [p, (i, b, c)] contiguously
    u = np.asarray(signal, np.float32).reshape(2, NB, BL, NCORES, C)
    in_maps = [{"w": wpack} for _ in range(NCORES)]
    b0 = 0
    for j, W in enumerate(WLIST):
        Xc = (u[:, b0:b0 + W].transpose(3, 2, 0, 1, 4)
              .reshape(NCORES, BL, 2 * W * C))
        for k in range(NCORES):
            in_maps[k][f"x{j}"] = Xc[k]
        b0 += W

    res = run_bass_kernel_spmd(nc, in_maps, core_ids=list(range(NCORES)),
                               trace=TRACE)
    LAST_RESULTS = res

    out = np.empty((T, B), np.float32)
    ob = out.reshape(NB, BL, NCORES, C)
    b0 = 0
    for j, W in enumerate(WLIST):
        for k in range(NCORES):
            ob[b0:b0 + W, :, k, :] = (res.results[k][f"y{j}"]
                                      .reshape(BL, W, C).transpose(1, 0, 2))
        b0 += W
    return out
